# revision 54
# baseline (speedup 1.0000x reference)
"""Causal self-attention (B=2, L=2048, C=2048, H=16) on 8 trn2 NeuronCores.

Sharding: tensor-parallel over heads — 2 heads per core. Each core computes
its heads' q/k/v from the full x, runs causal attention, and produces a
partial y @ w_proj[:, its-cols].T in fp16; the host sums the 8 partials.

Structure: phases are fused per 512-token chunk. Chunk ch computes the
q/k/v projections for tokens [512ch, 512ch+512), then immediately runs the
causal-attention items for q-group g=ch (all needed k-tiles are <= ch). The
output projection for a chunk is delayed by one chunk and its po chains are
interleaved into the next chunk's first qkv accumulation chain, absorbing
the x-DMA waits; so the PE stays ~95% occupied and its clock never ramps
down.

Notes:
- The reference's RoPE rotates q and k by identical per-head (position-
  independent) angles; an orthogonal rotation applied to both sides leaves
  q.k unchanged, so RoPE is skipped entirely.
- All matmul operands are bf16 (x/weights cast on host, halving DMA);
  accumulation stays fp32 in PSUM.
- Softmax skips the max-subtraction (safe score range) and computes the
  denominator on the PE: below-diagonal P tiles are cast to fp8 pairs and
  summed by fp8 DoubleRow matmuls (2x rate, 256-deep contraction); diagonal
  tiles use a bf16 ones-matmul. Both paths scale by 1/64 because the
  hardware fp8e4 saturates at 240 (IEEE e4m3 with inf, not e4m3fn); the 64x
  is folded into w_proj on the host. The [128,512] denominator psum rows are
  all equal, so the reciprocal broadcasts for free (no gpsimd broadcast).
- Output partials are written in fp16 (halves the output DMA); the final
  chunk's writes fan out across all three DMA queues to shorten the drain.
"""
import sys
sys.path.insert(0, '/opt/trn_rl_repo')
import contextlib
import ctypes
import os
import types

import numpy as np
import ml_dtypes

import concourse.bacc as bacc
import concourse.tile as tile
from concourse import mybir
from concourse.bass_utils import run_bass_kernel_spmd

F32 = mybir.dt.float32
F32R = mybir.dt.float32r
BF16 = mybir.dt.bfloat16
F16 = mybir.dt.float16
F8E4 = mybir.dt.float8e4
DR = mybir.MatmulPerfMode.DoubleRow
AF = mybir.ActivationFunctionType

B, L, C, H, D = 2, 2048, 2048, 16, 128
NCORES = 8
HPC = H // NCORES            # heads per core
TC = 512                     # token chunk == q-group width
NCH = L // TC                # chunks per batch (4)
KT16 = C // 128              # contraction tiles over C (16)
SCALE = 1.0 / float(np.sqrt(D))

LAST_RESULT = None           # BassKernelResults of the most recent run


def _install_ntff_shim():
    """Register the axon NTFF profile hook so BASS_TRACE=1 yields exec_time_ns."""
    if "antenv.axon_hooks" in sys.modules:
        return
    so_path = "/opt/axon/libaxon_pjrt.so"
    if not os.path.exists(so_path):
        return
    lib = ctypes.CDLL(so_path)
    if not hasattr(lib, "axon_start_nrt_profile"):
        return
    lib.axon_start_nrt_profile.argtypes = [ctypes.POINTER(ctypes.c_int64), ctypes.c_size_t]
    lib.axon_start_nrt_profile.restype = ctypes.c_int64
    lib.axon_stop_nrt_profile.argtypes = [ctypes.c_char_p]
    lib.axon_stop_nrt_profile.restype = ctypes.c_int64

    @contextlib.contextmanager
    def _hook(output_dir, device_ids):
        import jax
        jax.devices()
        if device_ids:
            ids = (ctypes.c_int64 * len(device_ids))(*device_ids)
            rc = lib.axon_start_nrt_profile(ids, len(device_ids))
        else:
            rc = lib.axon_start_nrt_profile(None, 0)
        if rc != 0:
            raise RuntimeError(f"axon_start_nrt_profile rc={rc}")
        try:
            yield
        finally:
            n = lib.axon_stop_nrt_profile(str(output_dir).encode())
            if n <= 0:
                print(f"ntff capture wrote {n} files to {output_dir}")

    mod = types.ModuleType("antenv.axon_hooks")
    mod.get_axon_ntff_profile_hook = lambda: _hook
    mod.set_axon_ntff_profile_hook = lambda h: None
    sys.modules["antenv.axon_hooks"] = mod


def _build():
    nc = bacc.Bacc()
    xT = nc.dram_tensor("xT", [C, B * L], BF16, kind="ExternalInput")
    wqkT = nc.dram_tensor("wqkT", [C, 4 * D], BF16, kind="ExternalInput")
    wvT = nc.dram_tensor("wvT", [C, 2 * D], BF16, kind="ExternalInput")
    wpT = nc.dram_tensor("wpT", [2 * D, C], BF16, kind="ExternalInput")
    maskd = nc.dram_tensor("maskd", [128, 4, 512], BF16, kind="ExternalInput")
    outd = nc.dram_tensor("out", [B * L, C], F16, kind="ExternalOutput")

    xR = xT.rearrange("(t p) n -> t p n", p=128)      # [16, 128, B*L]
    wqkR = wqkT.rearrange("(t p) n -> t p n", p=128)  # [16, 128, 512]
    wvR = wvT.rearrange("(t p) n -> t p n", p=128)    # [16, 128, 256]
    wpR = wpT.rearrange("(t p) n -> t p n", p=128)    # [2, 128, 2048]

    with tile.TileContext(nc) as tc:
        with tc.tile_pool(name="consts", bufs=1) as cp, \
             tc.tile_pool(name="big", bufs=1) as bp, \
             tc.tile_pool(name="xp", bufs=3) as xp, \
             tc.tile_pool(name="ptp", bufs=8) as ptp, \
             tc.tile_pool(name="p8p", bufs=4) as p8p, \
             tc.tile_pool(name="smp", bufs=3) as smp, \
             tc.tile_pool(name="osp", bufs=12) as osp, \
             tc.tile_pool(name="ps", bufs=4, space="PSUM") as ps, \
             tc.tile_pool(name="psa", bufs=2, space="PSUM") as psa:

            # PE pre-ramp on memset data while the first x/weight DMA
            # slices land (~10us).
            warm = cp.tile([128, 256], BF16)
            nc.vector.memset(warm, 0.0)
            pw = ps.tile([128, 256], F32, tag="mm")
            NWARM = 40
            for i in range(NWARM):
                nc.tensor.matmul(pw, warm[:, :128], warm,
                                 start=(i == 0), stop=(i == NWARM - 1))

            # Weights/constants stream on the ACT hwdge queue; the sync queue
            # is dedicated to x-chunk streaming. Order: q weights first (the
            # chunk-0 q chains run first), then wv, then k weights (first
            # needed by the S matmuls ~20us in), then wp/masks.
            wqk = cp.tile([128, KT16, 4 * D], BF16)
            wv = cp.tile([128, KT16, 2 * D], BF16)
            wp = cp.tile([128, HPC, C], BF16)
            wqkP = wqkR.transpose([1, 0, 2])  # [128, 16, 512]
            wvP = wvR.transpose([1, 0, 2])    # [128, 16, 256]

            def load_wqk(m):
                for k2 in range(KT16 // 2):
                    nc.scalar.dma_start(
                        out=wqk[:, 2 * k2:2 * k2 + 2, m * 128:(m + 1) * 128],
                        in_=wqkP[:, 2 * k2:2 * k2 + 2, m * 128:(m + 1) * 128])

            load_wqk(0)
            load_wqk(1)
            for k2 in range(KT16 // 2):
                nc.scalar.dma_start(out=wv[:, 2 * k2:2 * k2 + 2],
                                    in_=wvP[:, 2 * k2:2 * k2 + 2])
            load_wqk(2)
            load_wqk(3)
            for j in range(HPC):
                nc.scalar.dma_start(out=wp[:, j], in_=wpR[j])
            tm = cp.tile([128, 4, 512], BF16)
            nc.scalar.dma_start(out=tm, in_=maskd[:, :, :])
            # Denominator accumulates Z/64: exp values reach ~4300 and the
            # hardware fp8e4 tops out at 240 (IEEE e4m3 with inf, not e4m3fn),
            # so the fp8 copies of P are scaled by 1/64 (and the bf16 diagonal
            # path uses 1/64-valued ones). The 64x on y after normalization is
            # folded into w_proj on the host.
            tonesb = cp.tile([128, 128], BF16)
            nc.vector.memset(tonesb, 1.0 / 64.0)
            ones8 = cp.tile([128, 2, 128], F8E4)
            nc.vector.memset(ones8, 1.0)

            QT = bp.tile([128, HPC, L], BF16, tag="QT")   # [d, hi, tok]
            KT = bp.tile([128, HPC, L], BF16, tag="KT")
            V = bp.tile([128, L // 128, 2 * D], BF16, tag="V")  # [tok, tt, hi*D]
            yT = bp.tile([128, HPC, L], BF16, tag="yT")   # [d, hi, tok]

            def proj_one(pend, j):
                # One output-projection po chain (tokens of a chunk whose yT
                # norms are ~a chunk old). j indexes the 16 (tt, nch) pairs.
                b0, g0 = pend
                tt = 4 * g0 + j // 4
                nch = j % 4
                po = ps.tile([128, 512], F32, tag="mm")
                for hi in range(HPC):
                    nc.tensor.matmul(
                        po, yT[:, hi, tt * 128:(tt + 1) * 128],
                        wp[:, hi, nch * 512:(nch + 1) * 512],
                        start=(hi == 0), stop=(hi == HPC - 1),
                        skip_group_check=True)
                ot = osp.tile([128, 512], F16)
                nc.vector.tensor_copy(ot, po)
                # The final chunk's writes drain after the last matmul; spread
                # them across all three DMA queues (x streaming is done).
                if b0 == B - 1 and g0 == NCH - 1:
                    dq = (nc.gpsimd, nc.sync, nc.scalar)[j % 3]
                else:
                    dq = nc.gpsimd
                dq.dma_start(
                    out=outd[b0 * L + tt * 128: b0 * L + (tt + 1) * 128,
                             nch * 512:(nch + 1) * 512],
                    in_=ot)

            def proj_block(pend):
                for j in range(16):
                    proj_one(pend, j)

            pending_proj = None
            for b in range(B):
                for ch in range(NCH):
                    g = ch
                    t0 = b * L + ch * TC
                    # ---- x chunk DMA (split across the sync and DVE hwdge
                    # queues; per-k2 slices so the m chains can start as soon
                    # as the first slices land) ----
                    # The scalar (ACT) hwdge queue is busy with weights for
                    # the first ~2 chunks; after that split x across both
                    # queues for 2x issue rate.
                    # Split x slices across the sync and scalar queues once
                    # the weights have drained from the scalar queue.
                    xc = xp.tile([128, KT16, TC], BF16)
                    dual = b > 0 or ch >= 2
                    for k2 in range(KT16 // 2):
                        q = nc.scalar if (dual and k2 % 2 == 1) else nc.sync
                        q.dma_start(
                            out=xc[:, 2 * k2:2 * k2 + 2],
                            in_=xR.transpose([1, 0, 2])[:, 2 * k2:2 * k2 + 2,
                                                        t0:t0 + TC])

                    # ---- q/k/v projections for this chunk ----
                    # The m=0 chain races the x-chunk DMA; the previous
                    # chunk's proj po chains are interleaved into it (own
                    # psum groups) to absorb the per-slice DMA waits.
                    def m_chain(m, filler=None):
                        pq = ps.tile([128, TC], F32, tag="mm")
                        for k in range(KT16):
                            nc.tensor.matmul(pq, wqk[:, k, m * 128:(m + 1) * 128],
                                             xc[:, k], start=(k == 0),
                                             stop=(k == KT16 - 1),
                                             skip_group_check=(filler is not None))
                            if filler and k % 2 == 1 and k < 15:
                                filler.pop(0)()
                        dst = QT if m < 2 else KT
                        nc.vector.tensor_copy(dst[:, m % 2, ch * TC:(ch + 1) * TC], pq)

                    def v_chain(tt):
                        pv = ps.tile([128, 2 * D], F32, tag="mm")
                        for k in range(KT16):
                            nc.tensor.matmul(pv, xc[:, k, tt * 128:(tt + 1) * 128],
                                             wv[:, k], start=(k == 0),
                                             stop=(k == KT16 - 1))
                        nc.vector.tensor_copy(V[:, ch * (TC // 128) + tt], pv)

                    fillers = []
                    if pending_proj is not None:
                        pp = pending_proj
                        fillers = [(lambda j=j, pp=pp: proj_one(pp, j))
                                   for j in range(16)]
                    m_chain(0, fillers or None)
                    m_chain(1, fillers or None)
                    for tt in range(TC // 128):
                        v_chain(tt)
                    m_chain(2)
                    m_chain(3)
                    for f in fillers:
                        f()

                    # ---- causal attention for q-group g (tokens of this
                    # chunk) ----
                    # Item pipeline as in the baseline: S^T matmuls run a few
                    # items ahead; exp on ACT; diag tiles masked on DVE; PV
                    # and the ones (denominator) matmul accumulate on PE.
                    items = [(hi, kt)
                             for kt in range(4 * (g + 1))
                             for hi in range(HPC)]
                    nkt = 4 * (g + 1)

                    def s_matmul(hi, kt):
                        off = max(0, 128 * (kt - 4 * g))
                        pss = ps.tile([128, 512], F32, tag="mm")
                        nc.tensor.matmul(pss[:, off:],
                                         KT[:, hi, kt * 128:(kt + 1) * 128],
                                         QT[:, hi, g * 512 + off:(g + 1) * 512],
                                         start=True, stop=True)
                        return pss

                    pending_proj = (b, g)
                    pss_q = [s_matmul(*it) for it in items[:4]]

                    psy = {}
                    psr = {}
                    p8 = {}
                    for i, (hi, kt) in enumerate(items):
                        off = max(0, 128 * (kt - 4 * g))
                        if kt == 0:
                            psy[hi] = psa.tile([128, 512], F32, tag="acc",
                                               name=f"psy{hi}")
                            psr[hi] = psa.tile([128, 512], F32, tag="rs",
                                               name=f"psr{hi}")
                        pss = pss_q.pop(0)
                        ptile = ptp.tile([128, 512], BF16)
                        # exp/mask/PV run in 256-col halves so the PV of the
                        # first half overlaps ACT's exp of the second. Only
                        # the very first write to a psum bank carries
                        # start=True (it marks the whole 2KB region pending-
                        # zero; later halves land on pending-zero bytes).
                        halves = ([(off, 256), (256, 512)] if off < 256
                                  else [(off, 512)])
                        for nh, (lo, hn) in enumerate(halves):
                            nc.scalar.activation(ptile[:, lo:hn], pss[:, lo:hn],
                                                 AF.Exp, scale=SCALE)
                            if nh == 0 and i + 4 < len(items):
                                pss_q.append(s_matmul(*items[i + 4]))
                            if kt >= 4 * g:
                                nc.vector.tensor_mul(ptile[:, lo:hn],
                                                     ptile[:, lo:hn],
                                                     tm[:, kt - 4 * g, lo:hn])
                            nc.tensor.matmul(psy[hi][:, lo:hn],
                                             V[:, kt, hi * D:(hi + 1) * D],
                                             ptile[:, lo:hn],
                                             start=(kt == 0 and nh == 0),
                                             stop=(kt == nkt - 1
                                                   and nh == len(halves) - 1),
                                             skip_group_check=True)
                            if kt >= 4 * g:
                                nc.tensor.matmul(psr[hi][:, lo:hn], tonesb,
                                                 ptile[:, lo:hn],
                                                 start=(kt == 0 and g == 0
                                                        and nh == 0),
                                                 stop=(kt == nkt - 1
                                                       and nh == len(halves) - 1),
                                                 skip_group_check=True)
                        # Softmax denominator: full (below-diagonal) tiles are
                        # cast to fp8 pairs and summed by one DoubleRow matmul
                        # per pair (2x); diagonal tiles used the bf16 ones
                        # matmuls emitted per half above.
                        if kt < 4 * g:
                            if kt % 2 == 0:
                                p8[hi] = p8p.tile([128, 2, 512], F8E4,
                                                  name=f"p8_{hi}")
                            nc.vector.tensor_scalar_mul(p8[hi][:, kt % 2],
                                                        ptile, 1.0 / 64.0)
                            if kt % 2 == 1:
                                nc.tensor.matmul(psr[hi], ones8, p8[hi],
                                                 start=(kt == 1), stop=False,
                                                 perf_mode=DR,
                                                 skip_group_check=True)
                        if kt == nkt - 1:
                            rb = smp.tile([128, 512], F32, tag="rb")
                            nc.vector.reciprocal_approx_fast(out=rb, in_=psr[hi])
                            nc.vector.tensor_mul(yT[:, hi, g * 512:(g + 1) * 512],
                                                 psy[hi], rb)
            proj_block(pending_proj)
    nc.compile()
    return nc


def _make_masks():
    masks = np.zeros((128, 4, 512), dtype=np.float32)
    kk = np.arange(128)[:, None]
    qq = np.arange(128)[None, :]
    tri = (kk <= qq).astype(np.float32)
    for p in range(4):
        for j in range(4):
            blk = masks[:, p, j * 128:(j + 1) * 128]
            if j > p:
                blk[:] = 1.0
            elif j == p:
                blk[:] = tri
    return masks.astype(ml_dtypes.bfloat16)


_cached_nc = None


def kernel(x, w_attn, w_proj):
    global _cached_nc, LAST_RESULT
    if os.environ.get("BASS_TRACE"):
        _install_ntff_shim()
    if _cached_nc is None:
        _cached_nc = _build()
    nc = _cached_nc

    x = np.asarray(x, dtype=np.float32)
    w_attn = np.asarray(w_attn, dtype=np.float32)
    w_proj = np.asarray(w_proj, dtype=np.float32)

    xT = np.ascontiguousarray(x.reshape(B * L, C).T)
    masks = _make_masks()

    in_maps = []
    for c in range(NCORES):
        h0 = HPC * c
        wq = w_attn[h0 * D:(h0 + HPC) * D]
        wk = w_attn[C + h0 * D: C + (h0 + HPC) * D]
        wvs = w_attn[2 * C + h0 * D: 2 * C + (h0 + HPC) * D]
        in_maps.append({
            "xT": xT.astype(ml_dtypes.bfloat16),
            "wqkT": np.ascontiguousarray(
                np.concatenate([wq, wk], axis=0).T).astype(ml_dtypes.bfloat16),
            "wvT": np.ascontiguousarray(wvs.T).astype(ml_dtypes.bfloat16),
            "wpT": np.ascontiguousarray(
                w_proj[:, h0 * D:(h0 + HPC) * D].T / 64.0).astype(ml_dtypes.bfloat16),
            "maskd": masks,
        })

    res = run_bass_kernel_spmd(nc, in_maps, core_ids=list(range(NCORES)))
    LAST_RESULT = res
    acc = res.results[0]["out"].astype(np.float32)
    for i in range(1, NCORES):
        acc += res.results[i]["out"].astype(np.float32)
    return acc.reshape(B, L, C)


# revision 55
# speedup vs baseline: 1.0428x; 1.0428x over previous
"""Causal self-attention (B=2, L=2048, C=2048, H=16) on 8 trn2 NeuronCores.

Sharding: tensor-parallel over heads — 2 heads per core. Each core computes
its heads' q/k/v from the full x, runs causal attention, and produces a
partial y @ w_proj[:, its-cols].T in fp16; the host sums the 8 partials.

Structure: phases are fused per 512-token chunk. Chunk ch computes the
q/k/v projections for tokens [512ch, 512ch+512), then immediately runs the
causal-attention items for q-group g=ch (all needed k-tiles are <= ch). The
output projection for a chunk is delayed by one chunk and its po chains are
interleaved into the next chunk's first qkv accumulation chain, absorbing
the x-DMA waits; so the PE stays ~95% occupied and its clock never ramps
down.

Notes:
- The reference's RoPE rotates q and k by identical per-head (position-
  independent) angles; an orthogonal rotation applied to both sides leaves
  q.k unchanged, so RoPE is skipped entirely.
- All matmul operands are bf16 (x/weights cast on host, halving DMA);
  accumulation stays fp32 in PSUM.
- Softmax skips the max-subtraction (safe score range) and computes the
  denominator on the PE: below-diagonal P tiles are cast to fp8 pairs and
  summed by fp8 DoubleRow matmuls (2x rate, 256-deep contraction); diagonal
  tiles use a bf16 ones-matmul. Both paths scale by 1/64 because the
  hardware fp8e4 saturates at 240 (IEEE e4m3 with inf, not e4m3fn); the 64x
  is folded into w_proj on the host. The [128,512] denominator psum rows are
  all equal, so the reciprocal broadcasts for free (no gpsimd broadcast).
- Output partials are written in fp16 (halves the output DMA); the final
  chunk's writes fan out across all three DMA queues to shorten the drain.
"""
import sys
sys.path.insert(0, '/opt/trn_rl_repo')
import contextlib
import ctypes
import os
import types

import numpy as np
import ml_dtypes

import concourse.bacc as bacc
import concourse.tile as tile
from concourse import mybir
from concourse.bass_utils import run_bass_kernel_spmd

F32 = mybir.dt.float32
F32R = mybir.dt.float32r
BF16 = mybir.dt.bfloat16
F16 = mybir.dt.float16
F8E4 = mybir.dt.float8e4
DR = mybir.MatmulPerfMode.DoubleRow
AF = mybir.ActivationFunctionType

B, L, C, H, D = 2, 2048, 2048, 16, 128
NCORES = 8
HPC = H // NCORES            # heads per core
TC = 512                     # token chunk == q-group width
NCH = L // TC                # chunks per batch (4)
KT16 = C // 128              # contraction tiles over C (16)
SCALE = 1.0 / float(np.sqrt(D))

LAST_RESULT = None           # BassKernelResults of the most recent run


def _install_ntff_shim():
    """Register the axon NTFF profile hook so BASS_TRACE=1 yields exec_time_ns."""
    if "antenv.axon_hooks" in sys.modules:
        return
    so_path = "/opt/axon/libaxon_pjrt.so"
    if not os.path.exists(so_path):
        return
    lib = ctypes.CDLL(so_path)
    if not hasattr(lib, "axon_start_nrt_profile"):
        return
    lib.axon_start_nrt_profile.argtypes = [ctypes.POINTER(ctypes.c_int64), ctypes.c_size_t]
    lib.axon_start_nrt_profile.restype = ctypes.c_int64
    lib.axon_stop_nrt_profile.argtypes = [ctypes.c_char_p]
    lib.axon_stop_nrt_profile.restype = ctypes.c_int64

    @contextlib.contextmanager
    def _hook(output_dir, device_ids):
        import jax
        jax.devices()
        if device_ids:
            ids = (ctypes.c_int64 * len(device_ids))(*device_ids)
            rc = lib.axon_start_nrt_profile(ids, len(device_ids))
        else:
            rc = lib.axon_start_nrt_profile(None, 0)
        if rc != 0:
            raise RuntimeError(f"axon_start_nrt_profile rc={rc}")
        try:
            yield
        finally:
            n = lib.axon_stop_nrt_profile(str(output_dir).encode())
            if n <= 0:
                print(f"ntff capture wrote {n} files to {output_dir}")

    mod = types.ModuleType("antenv.axon_hooks")
    mod.get_axon_ntff_profile_hook = lambda: _hook
    mod.set_axon_ntff_profile_hook = lambda h: None
    sys.modules["antenv.axon_hooks"] = mod


def _build():
    nc = bacc.Bacc()
    xT = nc.dram_tensor("xT", [C, B * L], BF16, kind="ExternalInput")
    wqkT = nc.dram_tensor("wqkT", [C, 4 * D], BF16, kind="ExternalInput")
    wvT = nc.dram_tensor("wvT", [C, 2 * D], BF16, kind="ExternalInput")
    wpT = nc.dram_tensor("wpT", [2 * D, C], BF16, kind="ExternalInput")
    maskd = nc.dram_tensor("maskd", [128, 4, 512], BF16, kind="ExternalInput")
    outd = nc.dram_tensor("out", [B * L, C], F16, kind="ExternalOutput")

    xR = xT.rearrange("(t p) n -> t p n", p=128)      # [16, 128, B*L]
    wqkR = wqkT.rearrange("(t p) n -> t p n", p=128)  # [16, 128, 512]
    wvR = wvT.rearrange("(t p) n -> t p n", p=128)    # [16, 128, 256]
    wpR = wpT.rearrange("(t p) n -> t p n", p=128)    # [2, 128, 2048]

    with tile.TileContext(nc) as tc:
        with tc.tile_pool(name="consts", bufs=1) as cp, \
             tc.tile_pool(name="big", bufs=1) as bp, \
             tc.tile_pool(name="xp", bufs=3) as xp, \
             tc.tile_pool(name="ptp", bufs=8) as ptp, \
             tc.tile_pool(name="p8p", bufs=4) as p8p, \
             tc.tile_pool(name="smp", bufs=3) as smp, \
             tc.tile_pool(name="osp", bufs=12) as osp, \
             tc.tile_pool(name="ps", bufs=4, space="PSUM") as ps, \
             tc.tile_pool(name="psa", bufs=2, space="PSUM") as psa:

            # PE pre-ramp on memset data while the first x/weight DMA
            # slices land (~10us).
            warm = cp.tile([128, 256], BF16)
            nc.vector.memset(warm, 0.0)
            pw = ps.tile([128, 256], F32, tag="mm")
            NWARM = 40
            for i in range(NWARM):
                nc.tensor.matmul(pw, warm[:, :128], warm,
                                 start=(i == 0), stop=(i == NWARM - 1))

            # Weights/constants stream on the ACT hwdge queue; the sync queue
            # is dedicated to x-chunk streaming. Order: q weights first (the
            # chunk-0 q chains run first), then wv, then k weights (first
            # needed by the S matmuls ~20us in), then wp/masks.
            wqk = cp.tile([128, KT16, 4 * D], BF16)
            wv = cp.tile([128, KT16, 2 * D], BF16)
            wp = cp.tile([128, HPC, C], BF16)
            wqkP = wqkR.transpose([1, 0, 2])  # [128, 16, 512]
            wvP = wvR.transpose([1, 0, 2])    # [128, 16, 256]

            def load_wqk(m):
                for k2 in range(KT16 // 2):
                    nc.scalar.dma_start(
                        out=wqk[:, 2 * k2:2 * k2 + 2, m * 128:(m + 1) * 128],
                        in_=wqkP[:, 2 * k2:2 * k2 + 2, m * 128:(m + 1) * 128])

            load_wqk(0)
            load_wqk(1)
            for k2 in range(KT16 // 2):
                nc.scalar.dma_start(out=wv[:, 2 * k2:2 * k2 + 2],
                                    in_=wvP[:, 2 * k2:2 * k2 + 2])
            load_wqk(2)
            load_wqk(3)
            for j in range(HPC):
                nc.scalar.dma_start(out=wp[:, j], in_=wpR[j])
            tm = cp.tile([128, 4, 512], BF16)
            nc.scalar.dma_start(out=tm, in_=maskd[:, :, :])
            # Denominator accumulates Z/64: exp values reach ~4300 and the
            # hardware fp8e4 tops out at 240 (IEEE e4m3 with inf, not e4m3fn),
            # so the fp8 copies of P are scaled by 1/64 (and the bf16 diagonal
            # path uses 1/64-valued ones). The 64x on y after normalization is
            # folded into w_proj on the host.
            tonesb = cp.tile([128, 128], BF16)
            nc.vector.memset(tonesb, 1.0 / 64.0)
            ones8 = cp.tile([128, 2, 128], F8E4)
            nc.vector.memset(ones8, 1.0)

            QT = bp.tile([128, HPC, L], BF16, tag="QT")   # [d, hi, tok]
            KT = bp.tile([128, HPC, L], BF16, tag="KT")
            V = bp.tile([128, L // 128, 2 * D], BF16, tag="V")  # [tok, tt, hi*D]
            yT = bp.tile([128, HPC, L], BF16, tag="yT")   # [d, hi, tok]

            def proj_one(pend, j):
                # One output-projection po chain (tokens of a chunk whose yT
                # norms are ~a chunk old). j indexes the 16 (tt, nch) pairs.
                b0, g0 = pend
                tt = 4 * g0 + j // 4
                nch = j % 4
                po = ps.tile([128, 512], F32, tag="mm")
                for hi in range(HPC):
                    nc.tensor.matmul(
                        po, yT[:, hi, tt * 128:(tt + 1) * 128],
                        wp[:, hi, nch * 512:(nch + 1) * 512],
                        start=(hi == 0), stop=(hi == HPC - 1),
                        skip_group_check=True)
                ot = osp.tile([128, 512], F16)
                nc.vector.tensor_copy(ot, po)
                # The final chunk's writes drain after the last matmul; spread
                # them across all three DMA queues (x streaming is done).
                if b0 == B - 1 and g0 == NCH - 1:
                    dq = (nc.gpsimd, nc.sync, nc.scalar)[j % 3]
                else:
                    dq = nc.gpsimd
                dq.dma_start(
                    out=outd[b0 * L + tt * 128: b0 * L + (tt + 1) * 128,
                             nch * 512:(nch + 1) * 512],
                    in_=ot)

            def proj_block(pend):
                for j in range(16):
                    proj_one(pend, j)

            pending_proj = None
            for b in range(B):
                for ch in range(NCH):
                    g = ch
                    t0 = b * L + ch * TC
                    # ---- x chunk DMA (split across the sync and DVE hwdge
                    # queues; per-k2 slices so the m chains can start as soon
                    # as the first slices land) ----
                    # The scalar (ACT) hwdge queue is busy with weights for
                    # the first ~2 chunks; after that split x across both
                    # queues for 2x issue rate.
                    # Split x slices across the sync and scalar queues once
                    # the weights have drained from the scalar queue.
                    xc = xp.tile([128, KT16, TC], BF16)
                    dual = b > 0 or ch >= 2
                    for k2 in range(KT16 // 2):
                        q = nc.scalar if (dual and k2 % 2 == 1) else nc.sync
                        q.dma_start(
                            out=xc[:, 2 * k2:2 * k2 + 2],
                            in_=xR.transpose([1, 0, 2])[:, 2 * k2:2 * k2 + 2,
                                                        t0:t0 + TC])

                    # ---- q/k/v projections for this chunk ----
                    # The m=0 chain races the x-chunk DMA; the previous
                    # chunk's proj po chains are interleaved into it (own
                    # psum groups) to absorb the per-slice DMA waits.
                    def m_chain(m, filler=None):
                        pq = ps.tile([128, TC], F32, tag="mm")
                        for k in range(KT16):
                            nc.tensor.matmul(pq, wqk[:, k, m * 128:(m + 1) * 128],
                                             xc[:, k], start=(k == 0),
                                             stop=(k == KT16 - 1),
                                             skip_group_check=(filler is not None))
                            if filler and k % 2 == 1 and k < 15:
                                filler.pop(0)()
                        dst = QT if m < 2 else KT
                        nc.vector.tensor_copy(dst[:, m % 2, ch * TC:(ch + 1) * TC], pq)

                    def v_chain(tt):
                        pv = ps.tile([128, 2 * D], F32, tag="mm")
                        for k in range(KT16):
                            nc.tensor.matmul(pv, xc[:, k, tt * 128:(tt + 1) * 128],
                                             wv[:, k], start=(k == 0),
                                             stop=(k == KT16 - 1))
                        nc.vector.tensor_copy(V[:, ch * (TC // 128) + tt], pv)

                    fillers = []
                    if pending_proj is not None:
                        pp = pending_proj
                        fillers = [(lambda j=j, pp=pp: proj_one(pp, j))
                                   for j in range(16)]
                    m_chain(0, fillers or None)
                    m_chain(1, fillers or None)
                    for tt in range(TC // 128):
                        v_chain(tt)
                    m_chain(2)
                    m_chain(3)
                    for f in fillers:
                        f()

                    # ---- causal attention for q-group g (tokens of this
                    # chunk) ----
                    # Item pipeline as in the baseline: S^T matmuls run a few
                    # items ahead; exp on ACT; diag tiles masked on DVE; PV
                    # and the ones (denominator) matmul accumulate on PE.
                    items = [(hi, kt)
                             for kt in range(4 * (g + 1))
                             for hi in range(HPC)]
                    nkt = 4 * (g + 1)

                    def s_matmul(hi, kt):
                        off = max(0, 128 * (kt - 4 * g))
                        pss = ps.tile([128, 512], F32, tag="mm")
                        nc.tensor.matmul(pss[:, off:],
                                         KT[:, hi, kt * 128:(kt + 1) * 128],
                                         QT[:, hi, g * 512 + off:(g + 1) * 512],
                                         start=True, stop=True)
                        return pss

                    pending_proj = (b, g)
                    pss_q = [s_matmul(*it) for it in items[:4]]

                    psy = {}
                    psr = {}
                    p8 = {}
                    for i, (hi, kt) in enumerate(items):
                        off = max(0, 128 * (kt - 4 * g))
                        if kt == 0:
                            psy[hi] = psa.tile([128, 512], F32, tag="acc",
                                               name=f"psy{hi}")
                            psr[hi] = psa.tile([128, 512], F32, tag="rs",
                                               name=f"psr{hi}")
                        pss = pss_q.pop(0)
                        ptile = ptp.tile([128, 512], BF16)
                        nc.scalar.activation(ptile[:, off:], pss[:, off:],
                                             AF.Exp, scale=SCALE)
                        if i + 4 < len(items):
                            pss_q.append(s_matmul(*items[i + 4]))
                        if kt >= 4 * g:
                            nc.vector.tensor_mul(ptile[:, off:], ptile[:, off:],
                                                 tm[:, kt - 4 * g, off:])
                        nc.tensor.matmul(psy[hi][:, off:],
                                         V[:, kt, hi * D:(hi + 1) * D],
                                         ptile[:, off:],
                                         start=(kt == 0), stop=(kt == nkt - 1),
                                         skip_group_check=True)
                        if kt >= 4 * g:
                            nc.tensor.matmul(psr[hi][:, off:], tonesb,
                                             ptile[:, off:],
                                             start=(kt == 0 and g == 0),
                                             stop=(kt == nkt - 1),
                                             skip_group_check=True)
                        # Softmax denominator: full (below-diagonal) tiles are
                        # cast to fp8 pairs and summed by one DoubleRow matmul
                        # per pair (2x); diagonal tiles used the bf16 ones
                        # matmuls emitted per half above.
                        if kt < 4 * g:
                            if kt % 2 == 0:
                                p8[hi] = p8p.tile([128, 2, 512], F8E4,
                                                  name=f"p8_{hi}")
                            nc.vector.tensor_scalar_mul(p8[hi][:, kt % 2],
                                                        ptile, 1.0 / 64.0)
                            if kt % 2 == 1:
                                nc.tensor.matmul(psr[hi], ones8, p8[hi],
                                                 start=(kt == 1), stop=False,
                                                 perf_mode=DR,
                                                 skip_group_check=True)
                        if kt == nkt - 1:
                            rb = smp.tile([128, 512], F32, tag="rb")
                            nc.vector.reciprocal_approx_fast(out=rb, in_=psr[hi])
                            nc.vector.tensor_mul(yT[:, hi, g * 512:(g + 1) * 512],
                                                 psy[hi], rb)
            proj_block(pending_proj)
    nc.compile()
    return nc


def _make_masks():
    masks = np.zeros((128, 4, 512), dtype=np.float32)
    kk = np.arange(128)[:, None]
    qq = np.arange(128)[None, :]
    tri = (kk <= qq).astype(np.float32)
    for p in range(4):
        for j in range(4):
            blk = masks[:, p, j * 128:(j + 1) * 128]
            if j > p:
                blk[:] = 1.0
            elif j == p:
                blk[:] = tri
    return masks.astype(ml_dtypes.bfloat16)


_cached_nc = None


def kernel(x, w_attn, w_proj):
    global _cached_nc, LAST_RESULT
    if os.environ.get("BASS_TRACE"):
        _install_ntff_shim()
    if _cached_nc is None:
        _cached_nc = _build()
    nc = _cached_nc

    x = np.asarray(x, dtype=np.float32)
    w_attn = np.asarray(w_attn, dtype=np.float32)
    w_proj = np.asarray(w_proj, dtype=np.float32)

    xT = np.ascontiguousarray(x.reshape(B * L, C).T)
    masks = _make_masks()

    in_maps = []
    for c in range(NCORES):
        h0 = HPC * c
        wq = w_attn[h0 * D:(h0 + HPC) * D]
        wk = w_attn[C + h0 * D: C + (h0 + HPC) * D]
        wvs = w_attn[2 * C + h0 * D: 2 * C + (h0 + HPC) * D]
        in_maps.append({
            "xT": xT.astype(ml_dtypes.bfloat16),
            "wqkT": np.ascontiguousarray(
                np.concatenate([wq, wk], axis=0).T).astype(ml_dtypes.bfloat16),
            "wvT": np.ascontiguousarray(wvs.T).astype(ml_dtypes.bfloat16),
            "wpT": np.ascontiguousarray(
                w_proj[:, h0 * D:(h0 + HPC) * D].T / 64.0).astype(ml_dtypes.bfloat16),
            "maskd": masks,
        })

    res = run_bass_kernel_spmd(nc, in_maps, core_ids=list(range(NCORES)))
    LAST_RESULT = res
    acc = res.results[0]["out"].astype(np.float32)
    for i in range(1, NCORES):
        acc += res.results[i]["out"].astype(np.float32)
    return acc.reshape(B, L, C)
